# revision 25
# baseline (speedup 1.0000x reference)
"""Trainium2 Bass kernel for nn_CrystalDiffusionModel (gnn_message_passing).

Strategy (graph/edge parallelism, 8 cores):
  - Nodes are globally sorted by degree-bucket d = ceil(deg/8) and dealt
    round-robin to cores, so every core has the same canonical chunk/reduce
    structure (single SPMD program; per-core differences are data only).
  - All linear layers / biases are folded host-side into per-conv (A, B, beta,
    W2); the device runs 8 identical edge-conv layers:
        h_l = segmax_{e in n} relu(h[dst_e] @ A + h[src_e] @ B + beta) @ W2
  - Per layer each core holds the full node table h (bf16) in SBUF in a
    token-major layout; per 1024-edge chunk it does two SBUF-source
    transposed dma_gathers (x_j per edge, x_i per group-of-8 dst), two PE
    matmuls accumulating into PSUM (the group x_i is expanded 8x with a
    0-stride access pattern), an ACT relu+bias evac, a PE W2 matmul, and a
    DVE segmented reduce_max straight out of PSUM into the feature-major
    h-columns of the owned nodes.
  - Layer tail: 10 PE transposes -> bf16 rows -> AllGather -> reload table.
  - Finale: score = h8 @ decfc_w per owned node, host adds the folded bias,
    unpermutes, and computes mean((score - noise)^2) in float64.
"""
import sys

for _p in ("/opt/trn_rl_repo",):
    if _p not in sys.path:
        sys.path.insert(0, _p)

import numpy as np
import ml_dtypes

bf16 = ml_dtypes.bfloat16

N = 10000
E = 320000
H = 128
T_STEPS = 1000
NCORES = 8
CHUNK = 1024
GRP = 8
R_TILES = 10                  # 128-node tiles per core
TOK_PER_CORE = R_TILES * 128  # 1280


# --------------------------------------------------------------------------
# host-side planning
# --------------------------------------------------------------------------

def fold_weights(inp):
    f = lambda k: np.asarray(inp[k], np.float64)
    t = int(inp["t"])
    steps = np.linspace(0.0, float(T_STEPS), T_STEPS + 1)
    s = 0.008
    alpha_bar = np.cos((steps / T_STEPS + s) / (1.0 + s) * (np.pi / 2)) ** 2
    ab_t = alpha_bar[t]
    a_coef, b_coef = np.sqrt(ab_t), np.sqrt(1.0 - ab_t)
    tf = t / T_STEPS
    freq = np.exp(np.linspace(-4.0, 4.0, 32))
    emb = np.concatenate([np.sin(tf * freq), np.cos(tf * freq)])
    t_emb = emb @ f("tp_w") + f("tp_b")

    w1 = [f("enc_w1")[i] for i in range(4)] + [f("dec_w1")[i] for i in range(4)]
    b1 = [f("enc_b1")[i] for i in range(4)] + [f("dec_b1")[i] for i in range(4)]
    w2 = [f("enc_w2")[i] for i in range(4)] + [f("dec_w2")[i] for i in range(4)]
    b2 = [f("enc_b2")[i] for i in range(4)] + [f("dec_b2")[i] for i in range(4)]

    A, B, beta = [], [], []
    for l in range(8):
        Al = w1[l][:H] - w1[l][H:]
        Bl = w1[l][H:]
        if l == 0:
            W, c = f("emb_w"), t_emb @ f("emb_w") + f("emb_b")
        elif l == 4:
            W, c = f("encfc_w"), b2[3] @ f("encfc_w") + f("encfc_b")
        else:
            W, c = np.eye(H), b2[l - 1]
        A.append(W @ Al)
        B.append(W @ Bl)
        beta.append(b1[l] + c @ (Al + Bl))
    Wf = f("decfc_w")
    cf = b2[7] @ f("decfc_w") + f("decfc_b")
    return dict(A=A, B=B, beta=beta, W2=w2, Wf=Wf, cf=cf,
                a_coef=a_coef, b_coef=b_coef)


def build_plan(edge_index, ncores):
    """Canonical (SPMD-uniform) chunk plan + per-core gather streams."""
    src = np.asarray(edge_index[0], np.int64)
    dst = np.asarray(edge_index[1], np.int64)
    loop = np.arange(N, dtype=np.int64)
    src = np.concatenate([src, loop])
    dst = np.concatenate([dst, loop])

    deg = np.bincount(dst, minlength=N)
    assert deg.min() >= 1 and deg.max() <= 64, (deg.min(), deg.max())
    d_node = ((deg + GRP - 1) // GRP).astype(np.int64)  # 1..8

    order = np.argsort(dst, kind="stable")
    src_by_dst = src[order]
    starts = np.zeros(N + 1, np.int64)
    starts[1:] = np.cumsum(deg)

    # global bucket sort, deal round-robin
    gsort = np.argsort(d_node, kind="stable")
    core_nodes = [gsort[c::ncores] for c in range(ncores)]  # each sorted by d
    # canonical bucket counts = max over cores
    cnt_d = np.zeros(9, np.int64)
    for c in range(ncores):
        bc = np.bincount(d_node[core_nodes[c]], minlength=9)
        cnt_d = np.maximum(cnt_d, bc)
    # canonical node sequence: for d=1..8, cnt_d[d] slots
    canon_d = np.concatenate([np.full(cnt_d[d], d, np.int64) for d in range(1, 9)])
    ncols = len(canon_d)
    assert ncols <= TOK_PER_CORE, ncols

    # pack canonical sequence into chunks
    calls = []            # per chunk: list of (col0, cnt, d, edge_off)
    slot_of_col = []      # (chunk, edge_off) per canonical col
    cur_calls, cur = [], 0
    run0 = rund = runcnt = None
    nchunk = 0

    def flush_run(col_next):
        nonlocal runcnt
        if runcnt:
            cur_calls.append((run0, runcnt, rund, run_off))
        runcnt = 0

    runcnt = 0
    run_off = 0
    for col in range(ncols):
        d = int(canon_d[col])
        seg = d * GRP
        if cur + seg > CHUNK:
            flush_run(col)
            calls.append(cur_calls)
            cur_calls, cur = [], 0
            nchunk += 1
        if runcnt and rund != d:
            flush_run(col)
        if runcnt == 0:
            run0, rund, run_off = col, d, cur
        slot_of_col.append((nchunk, cur))
        runcnt += 1
        cur += seg
    flush_run(ncols)
    calls.append(cur_calls)
    nchunk += 1

    # token numbering: token_gidx(core c, col) = (c*R + col//128)*128 + col%128
    gidx_of_node = np.zeros(N, np.int64)
    col_of_node = np.zeros(N, np.int64)
    for c in range(ncores):
        nodes = core_nodes[c]
        # per-core col assignment: bucket-d nodes fill the canonical d-range
        # in order
        cols = np.zeros(len(nodes), np.int64)
        off = 0
        base = 0
        for d in range(1, 9):
            nd = int((d_node[nodes] == d).sum())
            cols[off:off + nd] = base + np.arange(nd)
            off += nd
            base += cnt_d[d]
        col_of_node[nodes] = cols
        gidx_of_node[nodes] = (c * R_TILES + cols // 128) * 128 + cols % 128

    # streams per core
    src_streams, xi_streams = [], []
    for c in range(ncores):
        nodes = core_nodes[c]
        ss = np.zeros(nchunk * CHUNK, np.int16)
        xs = np.zeros(nchunk * (CHUNK // GRP), np.int16)
        for n in nodes:
            col = col_of_node[n]
            k, off = slot_of_col[col]
            d = int(d_node[n])
            seg = d * GRP
            adj = gidx_of_node[src_by_dst[starts[n]:starts[n + 1]]]
            if len(adj) < seg:
                adj = np.concatenate([adj, np.full(seg - len(adj), adj[-1])])
            ss[k * CHUNK + off: k * CHUNK + off + seg] = adj
            g0 = k * (CHUNK // GRP) + off // GRP
            xs[g0:g0 + d] = gidx_of_node[n]
        src_streams.append(ss)
        xi_streams.append(xs)

    return dict(nchunk=nchunk, calls=calls, ncols=ncols,
                src_streams=src_streams, xi_streams=xi_streams,
                gidx_of_node=gidx_of_node, col_of_node=col_of_node,
                core_nodes=core_nodes)


def idx_sbuf_layout(idx):
    """int16 stream -> [128, n/16] wrapped (16-partition wrap, replicated x8)."""
    n = len(idx)
    assert n % 16 == 0
    arr = idx.reshape(n // 16, 16).T.astype(np.int16)  # [16, n/16]
    return np.ascontiguousarray(np.tile(arr, (8, 1)))  # [128, n/16]


def table_from_rows(rows, gidx, ncores):
    """rows [N,128] float -> table [128, ncores*R*128] bf16 addressed by gidx."""
    tbl = np.zeros((ncores * R_TILES * 128, H), bf16)
    tbl[gidx] = rows.astype(bf16)
    # token idx = stripe*128 + p  -> table[p, stripe*128:+128]
    t3 = tbl.reshape(ncores * R_TILES, 128, H).transpose(1, 0, 2)
    return np.ascontiguousarray(t3.reshape(128, ncores * R_TILES * H))


# --------------------------------------------------------------------------
# bass kernel
# --------------------------------------------------------------------------

def _patch_queue_aware_sems():
    """Partition Tile's DMASW semaphore lanes by SWDGE queue so dual-queue
    dma_gathers can't satisfy each other's completion waits."""
    import concourse.tile_sem_assignment as tsa
    import concourse.bass_isa as bass_isa
    import concourse.mybir as mybir
    if getattr(tsa.TileClockTick, "_qaware_patched", False):
        return
    _orig = tsa.TileClockTick._assign_tick

    def _assign_tick_qaware(self, inst):
        if (isinstance(inst, tsa.DMAInst)
                and not isinstance(inst, bass_isa.UserSyncedRemoteDMADescs)
                and inst.engine == mybir.EngineType.Pool):
            q = int(getattr(inst, "queue_num", 0) or 0) & 1
            ctrs = getattr(self, "_q_sw_ctr", None)
            if ctrs is None:
                ctrs = self._q_sw_ctr = {}
            half = max(self.swdge_sem_count // 2, 1)
            c = ctrs.get(q, 0)
            self.next_sw_dma_idx = (q * half + c % half) % self.swdge_sem_count
            ctrs[q] = c + 1
        return _orig(self, inst)

    tsa.TileClockTick._assign_tick = _assign_tick_qaware
    tsa.TileClockTick._qaware_patched = True


def build_nc(plan, ncores, layers=8, use_coll=True, nchunk_cap=None, repeat=1,
             nogather=False, singleq=False):
    import concourse.bass as bass
    import concourse.bacc as bacc
    import concourse.tile as tile
    import concourse.mybir as mybir
    from concourse.bass import ts
    _patch_queue_aware_sems()

    dt = mybir.dt
    AF = mybir.ActivationFunctionType
    nchunk = plan["nchunk"]
    if nchunk_cap is not None:
        nchunk = min(nchunk, nchunk_cap)
    calls = plan["calls"]
    NTOK = ncores * TOK_PER_CORE

    nc = bacc.Bacc("TRN2", target_bir_lowering=False, debug=False,
                   num_devices=ncores, num_swdge_queues=2)

    t0_d = nc.dram_tensor("t0", [128, NTOK], dt.bfloat16, kind="ExternalInput")
    src_d = nc.dram_tensor("srcidx", [128, nchunk * (CHUNK // 16)], dt.int16,
                           kind="ExternalInput")
    xi_d = nc.dram_tensor("xiidx", [128, nchunk * (CHUNK // GRP // 16)],
                          dt.int16, kind="ExternalInput")
    w_d = nc.dram_tensor("wts", [26, 128, 128], dt.bfloat16,
                         kind="ExternalInput")
    beta_d = nc.dram_tensor("betas", [128, 8], dt.float32,
                            kind="ExternalInput")
    score_d = nc.dram_tensor("score", [TOK_PER_CORE, 128], dt.float32,
                             kind="ExternalOutput")
    agouts = [nc.dram_tensor(f"agout{l}", [NTOK, 128], dt.bfloat16,
                             kind="Internal",
                             addr_space="Shared" if ncores > 4 else "Local")
              for l in range(7)]

    with tile.TileContext(nc) as tc:
        with (
            tc.tile_pool(name="const", bufs=1) as const,
            tc.tile_pool(name="tab", bufs=2) as tabp,
            tc.tile_pool(name="gat", bufs=4) as gpool,
            tc.tile_pool(name="act", bufs=4) as spool,
            tc.tile_pool(name="hfm", bufs=2) as hpool,
            tc.tile_pool(name="stg", bufs=2) as stgp,
            tc.tile_pool(name="ps", bufs=2, space="PSUM") as psp,
            tc.tile_pool(name="dram", bufs=2, space="DRAM") as dramp,
        ):
            wts = const.tile([128, 26 * 128], dt.bfloat16)
            nc.sync.dma_start(
                wts[:].rearrange("p (k e) -> p k e", k=26),
                w_d.ap().rearrange("k p e -> p k e"))
            betas = const.tile([128, 8], dt.float32)
            nc.sync.dma_start(betas[:], beta_d[:, :])
            srcidx = const.tile([128, nchunk * (CHUNK // 16)], dt.int16)
            nc.sync.dma_start(srcidx[:], src_d[:, :])
            xiidx = const.tile([128, nchunk * (CHUNK // GRP // 16)], dt.int16)
            nc.sync.dma_start(xiidx[:], xi_d[:, :])

            table = tabp.tile([128, NTOK], dt.bfloat16, tag="table")
            nc.sync.dma_start(table[:], t0_d[:, :])

            ident = lambda: wts[:, 25 * 128:26 * 128]
            wblk = lambda i: wts[:, i * 128:(i + 1) * 128]

            for li in range(8 - layers, 8 * repeat):
                l = li % 8
                last = li == 8 * repeat - 1
                A, B, W2 = wblk(3 * l), wblk(3 * l + 1), wblk(3 * l + 2)
                hfm = hpool.tile([128, TOK_PER_CORE], dt.bfloat16, tag="hfm")
                if plan["ncols"] < TOK_PER_CORE:
                    nc.vector.memset(hfm[:, plan["ncols"]:], 0.0)
                GB = 4  # chunks per xi-gather batch
                for g in range(0, nchunk, GB):
                    nb = min(GB, nchunk - g)
                    ngi = nb * (CHUNK // GRP)
                    xi = gpool.tile([128, GB * CHUNK // GRP], dt.bfloat16,
                                    tag="xi")
                    if nogather:
                        nc.vector.memset(xi[:], 0.125)
                    else:
                        nc.gpsimd.dma_gather(
                            xi[:, :ngi].rearrange("p (a b) -> p a b", a=1),
                            table[:],
                            xiidx[:, g * (CHUNK // GRP // 16):
                                  g * (CHUNK // GRP // 16) + ngi // 16],
                            num_idxs=ngi, num_idxs_reg=ngi, elem_size=H,
                            transpose=True, sbuf_tokens_per_rank=128,
                            sbuf_free_dim_per_rank=H * 2,
                            single_packet=False,
                            queue_num=0 if singleq else (g // GB) % 2,
                        )
                    for k in range(g, g + nb):
                        j0 = 0
                        i0 = (k - g) * (CHUNK // GRP)
                        xj = gpool.tile([128, CHUNK], dt.bfloat16, tag="xj")
                        if nogather:
                            nc.vector.memset(xj[:], 0.125)
                        else:
                            nc.gpsimd.dma_gather(
                                xj[:].rearrange("p (a b) -> p a b", a=1),
                                table[:],
                                srcidx[:, k * (CHUNK // 16):
                                       (k + 1) * (CHUNK // 16)],
                                num_idxs=CHUNK, num_idxs_reg=CHUNK,
                                elem_size=H,
                                transpose=True, sbuf_tokens_per_rank=128,
                                sbuf_free_dim_per_rank=H * 2,
                                single_packet=False,
                                queue_num=0 if singleq else k % 2,
                            )
                        pre = psp.tile([128, CHUNK], dt.float32, tag="pre")
                        for b in range(CHUNK // 512):
                            nc.tensor.matmul(pre[:, ts(b, 512)], B,
                                             xj[:, j0 + b * 512:
                                                j0 + (b + 1) * 512],
                                             start=True, stop=False)
                        for b in range(CHUNK // 512):
                            xi_b = (xi[:, i0 + b * 64:i0 + (b + 1) * 64]
                                    .unsqueeze(2)
                                    .broadcast_to([128, 64, GRP]))
                            nc.tensor.matmul(pre[:, ts(b, 512)], A, xi_b,
                                             start=False, stop=True)
                        s = spool.tile([128, CHUNK], dt.bfloat16, tag="s")
                        nc.scalar.activation(s[:], pre[:], AF.Relu,
                                             bias=betas[:, l:l + 1], scale=1.0)
                        msg = psp.tile([128, CHUNK], dt.float32, tag="msg")
                        for b in range(CHUNK // 512):
                            nc.tensor.matmul(msg[:, ts(b, 512)], W2,
                                             s[:, ts(b, 512)],
                                             start=True, stop=True)
                        for (col0, cnt, d, off) in calls[k]:
                            nc.vector.reduce_max(
                                hfm[:, col0:col0 + cnt],
                                msg[:, off:off + cnt * d * GRP]
                                .rearrange("p (n e) -> p n e", e=d * GRP),
                                axis=mybir.AxisListType.X)

                if not last:
                    stage = stgp.tile([128, TOK_PER_CORE], dt.bfloat16,
                                      tag="stage")
                    for t in range(R_TILES):
                        tp = psp.tile([128, 128], dt.bfloat16, tag="msg")
                        nc.tensor.transpose(tp[:], hfm[:, ts(t, 128)], ident())
                        nc.scalar.copy(stage[:, ts(t, 128)], tp[:])
                    agin = dramp.tile([TOK_PER_CORE, 128], dt.bfloat16,
                                      tag="agin")
                    nc.sync.dma_start(
                        agin[:].rearrange("(q t) e -> q (t e)", t=R_TILES),
                        stage[:])
                    table = tabp.tile([128, NTOK], dt.bfloat16, tag="table")
                    if use_coll:
                        agout = agouts[li % 7]
                        nc.gpsimd.collective_compute(
                            "AllGather", mybir.AluOpType.bypass,
                            replica_groups=[list(range(ncores))],
                            ins=[agin.opt()], outs=[agout.ap()])
                        nc.sync.dma_start(
                            table[:].rearrange("q (c t e) -> q c t e",
                                               c=ncores, t=R_TILES),
                            agout.ap().rearrange("(c q t) e -> q c t e",
                                                 c=ncores, t=R_TILES))
                    else:
                        # debug: local-only table refresh (wrong values,
                        # exercise-only)
                        nc.sync.dma_start(
                            table[:, :TOK_PER_CORE]
                            .rearrange("q (t e) -> q t e", t=R_TILES),
                            agin[:].rearrange("(q t) e -> q t e", t=R_TILES))
                        nc.vector.memset(table[:, TOK_PER_CORE:], 0.0)
                else:
                    stage = stgp.tile([128, TOK_PER_CORE], dt.float32,
                                      tag="fstage")
                    Wf = wblk(24)
                    for t in range(R_TILES):
                        tp = psp.tile([128, 128], dt.float32, tag="msg")
                        nc.tensor.matmul(tp[:], hfm[:, ts(t, 128)], Wf,
                                         start=True, stop=True)
                        nc.scalar.copy(stage[:, ts(t, 128)], tp[:])
                    nc.sync.dma_start(
                        score_d.ap().rearrange("(q t) e -> q (t e)",
                                               t=R_TILES),
                        stage[:])
    nc.compile()
    return nc


# --------------------------------------------------------------------------
# entry point
# --------------------------------------------------------------------------

def _prep_inputs(inputs, ncores):
    W = fold_weights(inputs)
    plan = build_plan(np.asarray(inputs["edge_index"]), ncores)

    x = np.asarray(inputs["x"], np.float64)
    noise = np.asarray(inputs["noise"], np.float64)
    x_noisy = (W["a_coef"] * x + W["b_coef"] * noise).astype(np.float32)

    t0 = table_from_rows(x_noisy, plan["gidx_of_node"], ncores)

    wts = np.zeros((26, 128, 128), bf16)
    for l in range(8):
        wts[3 * l] = W["A"][l].astype(bf16)
        wts[3 * l + 1] = W["B"][l].astype(bf16)
        wts[3 * l + 2] = W["W2"][l].astype(bf16)
    wts[24] = W["Wf"].astype(bf16)
    wts[25] = np.eye(128).astype(bf16)
    betas = np.stack([b.astype(np.float32) for b in W["beta"]], axis=1)

    in_maps = []
    for c in range(ncores):
        in_maps.append({
            "t0": t0,
            "srcidx": idx_sbuf_layout(plan["src_streams"][c]),
            "xiidx": idx_sbuf_layout(plan["xi_streams"][c]),
            "wts": wts,
            "betas": np.ascontiguousarray(betas),
        })
    return W, plan, in_maps


def _finish(results, W, plan, noise, ncores):
    """Per-core score tiles -> MSE (float64 host reduction)."""
    score = np.zeros((N, H), np.float64)
    for c in range(ncores):
        sc = results[c]["score"]  # [1280,128] rows q*10+t for col=128t+q
        nodes = plan["core_nodes"][c]
        cols = plan["col_of_node"][nodes]
        rows = (cols % 128) * R_TILES + cols // 128
        score[nodes] = sc[rows].astype(np.float64)
    score += W["cf"][None, :]
    return np.float32(np.mean((score - np.asarray(noise, np.float64)) ** 2))


_CACHE = {}


def run(inputs, trace=False, trace_kwargs=None):
    from concourse.bass_utils import run_bass_kernel_spmd

    ncores = NCORES
    W, plan, in_maps = _prep_inputs(inputs, ncores)

    key = (plan["nchunk"], plan["ncols"])
    if key not in _CACHE:
        _CACHE[key] = build_nc(plan, ncores)
    nc = _CACHE[key]

    res = run_bass_kernel_spmd(nc, in_maps, core_ids=list(range(ncores)),
                               trace=trace, **(trace_kwargs or {}))
    return _finish(res.results, W, plan, inputs["noise"], ncores), res


def kernel(**inputs) -> np.ndarray:
    return run(inputs)[0]


# revision 27
# speedup vs baseline: 1.1164x; 1.1164x over previous
"""Trainium2 Bass kernel for nn_CrystalDiffusionModel (gnn_message_passing).

Strategy (graph/edge parallelism, 8 cores):
  - Nodes are globally sorted by degree-bucket d = ceil(deg/8) and dealt
    round-robin to cores, so every core has the same canonical chunk/reduce
    structure (single SPMD program; per-core differences are data only).
  - All linear layers / biases are folded host-side into per-conv (A, B, beta,
    W2); the device runs 8 identical edge-conv layers:
        h_l = segmax_{e in n} relu(h[dst_e] @ A + h[src_e] @ B + beta) @ W2
  - Per layer each core holds the full node table h (bf16) in SBUF in a
    token-major layout; per 1024-edge chunk it does two SBUF-source
    transposed dma_gathers (x_j per edge, x_i per group-of-8 dst), two PE
    matmuls accumulating into PSUM (the group x_i is expanded 8x with a
    0-stride access pattern), an ACT relu+bias evac, a PE W2 matmul, and a
    DVE segmented reduce_max straight out of PSUM into the feature-major
    h-columns of the owned nodes.
  - Layer tail: 10 PE transposes -> bf16 rows -> AllGather -> reload table.
  - Finale: score = h8 @ decfc_w per owned node, host adds the folded bias,
    unpermutes, and computes mean((score - noise)^2) in float64.
"""
import sys

for _p in ("/opt/trn_rl_repo",):
    if _p not in sys.path:
        sys.path.insert(0, _p)

import numpy as np
import ml_dtypes

bf16 = ml_dtypes.bfloat16

N = 10000
E = 320000
H = 128
T_STEPS = 1000
NCORES = 8
CHUNK = 1024
GRP = 8
R_TILES = 10                  # 128-node tiles per core
TOK_PER_CORE = R_TILES * 128  # 1280


# --------------------------------------------------------------------------
# host-side planning
# --------------------------------------------------------------------------

def fold_weights(inp):
    f = lambda k: np.asarray(inp[k], np.float64)
    t = int(inp["t"])
    steps = np.linspace(0.0, float(T_STEPS), T_STEPS + 1)
    s = 0.008
    alpha_bar = np.cos((steps / T_STEPS + s) / (1.0 + s) * (np.pi / 2)) ** 2
    ab_t = alpha_bar[t]
    a_coef, b_coef = np.sqrt(ab_t), np.sqrt(1.0 - ab_t)
    tf = t / T_STEPS
    freq = np.exp(np.linspace(-4.0, 4.0, 32))
    emb = np.concatenate([np.sin(tf * freq), np.cos(tf * freq)])
    t_emb = emb @ f("tp_w") + f("tp_b")

    w1 = [f("enc_w1")[i] for i in range(4)] + [f("dec_w1")[i] for i in range(4)]
    b1 = [f("enc_b1")[i] for i in range(4)] + [f("dec_b1")[i] for i in range(4)]
    w2 = [f("enc_w2")[i] for i in range(4)] + [f("dec_w2")[i] for i in range(4)]
    b2 = [f("enc_b2")[i] for i in range(4)] + [f("dec_b2")[i] for i in range(4)]

    A, B, beta = [], [], []
    for l in range(8):
        Al = w1[l][:H] - w1[l][H:]
        Bl = w1[l][H:]
        if l == 0:
            W, c = f("emb_w"), t_emb @ f("emb_w") + f("emb_b")
        elif l == 4:
            W, c = f("encfc_w"), b2[3] @ f("encfc_w") + f("encfc_b")
        else:
            W, c = np.eye(H), b2[l - 1]
        A.append(W @ Al)
        B.append(W @ Bl)
        beta.append(b1[l] + c @ (Al + Bl))
    Wf = f("decfc_w")
    cf = b2[7] @ f("decfc_w") + f("decfc_b")
    return dict(A=A, B=B, beta=beta, W2=w2, Wf=Wf, cf=cf,
                a_coef=a_coef, b_coef=b_coef)


def build_plan(edge_index, ncores):
    """Canonical (SPMD-uniform) chunk plan + per-core gather streams."""
    src = np.asarray(edge_index[0], np.int64)
    dst = np.asarray(edge_index[1], np.int64)
    loop = np.arange(N, dtype=np.int64)
    src = np.concatenate([src, loop])
    dst = np.concatenate([dst, loop])

    deg = np.bincount(dst, minlength=N)
    assert deg.min() >= 1 and deg.max() <= 64, (deg.min(), deg.max())
    d_node = ((deg + GRP - 1) // GRP).astype(np.int64)  # 1..8

    order = np.argsort(dst, kind="stable")
    src_by_dst = src[order]
    starts = np.zeros(N + 1, np.int64)
    starts[1:] = np.cumsum(deg)

    # global bucket sort, deal round-robin
    gsort = np.argsort(d_node, kind="stable")
    core_nodes = [gsort[c::ncores] for c in range(ncores)]  # each sorted by d
    # canonical bucket counts = max over cores
    cnt_d = np.zeros(9, np.int64)
    for c in range(ncores):
        bc = np.bincount(d_node[core_nodes[c]], minlength=9)
        cnt_d = np.maximum(cnt_d, bc)
    # canonical node sequence: for d=1..8, cnt_d[d] slots
    canon_d = np.concatenate([np.full(cnt_d[d], d, np.int64) for d in range(1, 9)])
    ncols = len(canon_d)
    assert ncols <= TOK_PER_CORE, ncols

    # pack canonical sequence into chunks
    calls = []            # per chunk: list of (col0, cnt, d, edge_off)
    slot_of_col = []      # (chunk, edge_off) per canonical col
    cur_calls, cur = [], 0
    run0 = rund = runcnt = None
    nchunk = 0

    def flush_run(col_next):
        nonlocal runcnt
        if runcnt:
            cur_calls.append((run0, runcnt, rund, run_off))
        runcnt = 0

    runcnt = 0
    run_off = 0
    for col in range(ncols):
        d = int(canon_d[col])
        seg = d * GRP
        if cur + seg > CHUNK:
            flush_run(col)
            calls.append(cur_calls)
            cur_calls, cur = [], 0
            nchunk += 1
        if runcnt and rund != d:
            flush_run(col)
        if runcnt == 0:
            run0, rund, run_off = col, d, cur
        slot_of_col.append((nchunk, cur))
        runcnt += 1
        cur += seg
    flush_run(ncols)
    calls.append(cur_calls)
    nchunk += 1

    # token numbering: token_gidx(core c, col) = (c*R + col//128)*128 + col%128
    gidx_of_node = np.zeros(N, np.int64)
    col_of_node = np.zeros(N, np.int64)
    for c in range(ncores):
        nodes = core_nodes[c]
        # per-core col assignment: bucket-d nodes fill the canonical d-range
        # in order
        cols = np.zeros(len(nodes), np.int64)
        off = 0
        base = 0
        for d in range(1, 9):
            nd = int((d_node[nodes] == d).sum())
            cols[off:off + nd] = base + np.arange(nd)
            off += nd
            base += cnt_d[d]
        col_of_node[nodes] = cols
        gidx_of_node[nodes] = (c * R_TILES + cols // 128) * 128 + cols % 128

    # streams per core
    src_streams, xi_streams = [], []
    for c in range(ncores):
        nodes = core_nodes[c]
        ss = np.zeros(nchunk * CHUNK, np.int16)
        xs = np.zeros(nchunk * (CHUNK // GRP), np.int16)
        for n in nodes:
            col = col_of_node[n]
            k, off = slot_of_col[col]
            d = int(d_node[n])
            seg = d * GRP
            adj = gidx_of_node[src_by_dst[starts[n]:starts[n + 1]]]
            if len(adj) < seg:
                adj = np.concatenate([adj, np.full(seg - len(adj), adj[-1])])
            ss[k * CHUNK + off: k * CHUNK + off + seg] = adj
            g0 = k * (CHUNK // GRP) + off // GRP
            xs[g0:g0 + d] = gidx_of_node[n]
        src_streams.append(ss)
        xi_streams.append(xs)

    return dict(nchunk=nchunk, calls=calls, ncols=ncols,
                src_streams=src_streams, xi_streams=xi_streams,
                gidx_of_node=gidx_of_node, col_of_node=col_of_node,
                core_nodes=core_nodes)


def idx_sbuf_layout(idx):
    """int16 stream -> [128, n/16] wrapped (16-partition wrap, replicated x8)."""
    n = len(idx)
    assert n % 16 == 0
    arr = idx.reshape(n // 16, 16).T.astype(np.int16)  # [16, n/16]
    return np.ascontiguousarray(np.tile(arr, (8, 1)))  # [128, n/16]


def table_from_rows(rows, gidx, ncores):
    """rows [N,128] float -> table [128, ncores*R*128] bf16 addressed by gidx."""
    tbl = np.zeros((ncores * R_TILES * 128, H), bf16)
    tbl[gidx] = rows.astype(bf16)
    # token idx = stripe*128 + p  -> table[p, stripe*128:+128]
    t3 = tbl.reshape(ncores * R_TILES, 128, H).transpose(1, 0, 2)
    return np.ascontiguousarray(t3.reshape(128, ncores * R_TILES * H))


# --------------------------------------------------------------------------
# bass kernel
# --------------------------------------------------------------------------

def _patch_queue_aware_sems():
    """Partition Tile's DMASW semaphore lanes by SWDGE queue so dual-queue
    dma_gathers can't satisfy each other's completion waits."""
    import concourse.tile_sem_assignment as tsa
    import concourse.bass_isa as bass_isa
    import concourse.mybir as mybir
    if getattr(tsa.TileClockTick, "_qaware_patched", False):
        return
    _orig = tsa.TileClockTick._assign_tick

    def _assign_tick_qaware(self, inst):
        if (isinstance(inst, tsa.DMAInst)
                and not isinstance(inst, bass_isa.UserSyncedRemoteDMADescs)
                and inst.engine == mybir.EngineType.Pool):
            q = int(getattr(inst, "queue_num", 0) or 0) & 3
            ctrs = getattr(self, "_q_sw_ctr", None)
            if ctrs is None:
                ctrs = self._q_sw_ctr = {}
            part = max(self.swdge_sem_count // 4, 1)
            c = ctrs.get(q, 0)
            self.next_sw_dma_idx = (q * part + c % part) % self.swdge_sem_count
            ctrs[q] = c + 1
        return _orig(self, inst)

    tsa.TileClockTick._assign_tick = _assign_tick_qaware
    tsa.TileClockTick._qaware_patched = True


def build_nc(plan, ncores, layers=8, use_coll=True, nchunk_cap=None, repeat=1,
             nogather=False, singleq=False):
    import concourse.bass as bass
    import concourse.bacc as bacc
    import concourse.tile as tile
    import concourse.mybir as mybir
    from concourse.bass import ts
    _patch_queue_aware_sems()

    dt = mybir.dt
    AF = mybir.ActivationFunctionType
    nchunk = plan["nchunk"]
    if nchunk_cap is not None:
        nchunk = min(nchunk, nchunk_cap)
    calls = plan["calls"]
    NTOK = ncores * TOK_PER_CORE

    nc = bacc.Bacc("TRN2", target_bir_lowering=False, debug=False,
                   num_devices=ncores, num_swdge_queues=4)

    t0_d = nc.dram_tensor("t0", [128, NTOK], dt.bfloat16, kind="ExternalInput")
    src_d = nc.dram_tensor("srcidx", [128, nchunk * (CHUNK // 16)], dt.int16,
                           kind="ExternalInput")
    xi_d = nc.dram_tensor("xiidx", [128, nchunk * (CHUNK // GRP // 16)],
                          dt.int16, kind="ExternalInput")
    w_d = nc.dram_tensor("wts", [26, 128, 128], dt.bfloat16,
                         kind="ExternalInput")
    beta_d = nc.dram_tensor("betas", [128, 8], dt.float32,
                            kind="ExternalInput")
    score_d = nc.dram_tensor("score", [TOK_PER_CORE, 128], dt.float32,
                             kind="ExternalOutput")
    agouts = [nc.dram_tensor(f"agout{l}", [NTOK, 128], dt.bfloat16,
                             kind="Internal",
                             addr_space="Shared" if ncores > 4 else "Local")
              for l in range(7)]

    with tile.TileContext(nc) as tc:
        with (
            tc.tile_pool(name="const", bufs=1) as const,
            tc.tile_pool(name="tab", bufs=2) as tabp,
            tc.tile_pool(name="gat", bufs=4) as gpool,
            tc.tile_pool(name="act", bufs=4) as spool,
            tc.tile_pool(name="hfm", bufs=2) as hpool,
            tc.tile_pool(name="stg", bufs=2) as stgp,
            tc.tile_pool(name="ps", bufs=2, space="PSUM") as psp,
            tc.tile_pool(name="dram", bufs=2, space="DRAM") as dramp,
        ):
            wts = const.tile([128, 26 * 128], dt.bfloat16)
            nc.sync.dma_start(
                wts[:].rearrange("p (k e) -> p k e", k=26),
                w_d.ap().rearrange("k p e -> p k e"))
            betas = const.tile([128, 8], dt.float32)
            nc.sync.dma_start(betas[:], beta_d[:, :])
            srcidx = const.tile([128, nchunk * (CHUNK // 16)], dt.int16)
            nc.sync.dma_start(srcidx[:], src_d[:, :])
            xiidx = const.tile([128, nchunk * (CHUNK // GRP // 16)], dt.int16)
            nc.sync.dma_start(xiidx[:], xi_d[:, :])

            table = tabp.tile([128, NTOK], dt.bfloat16, tag="table")
            nc.sync.dma_start(table[:], t0_d[:, :])

            ident = lambda: wts[:, 25 * 128:26 * 128]
            wblk = lambda i: wts[:, i * 128:(i + 1) * 128]

            qctr = [0]
            for li in range(8 - layers, 8 * repeat):
                l = li % 8
                last = li == 8 * repeat - 1
                A, B, W2 = wblk(3 * l), wblk(3 * l + 1), wblk(3 * l + 2)
                hfm = hpool.tile([128, TOK_PER_CORE], dt.bfloat16, tag="hfm")
                if plan["ncols"] < TOK_PER_CORE:
                    nc.vector.memset(hfm[:, plan["ncols"]:], 0.0)
                GB = 4  # chunks per xi-gather batch
                for g in range(0, nchunk, GB):
                    nb = min(GB, nchunk - g)
                    ngi = nb * (CHUNK // GRP)
                    xi = gpool.tile([128, GB * CHUNK // GRP], dt.bfloat16,
                                    tag="xi")
                    if nogather:
                        nc.vector.memset(xi[:], 0.125)
                    else:
                        nc.gpsimd.dma_gather(
                            xi[:, :ngi].rearrange("p (a b) -> p a b", a=1),
                            table[:],
                            xiidx[:, g * (CHUNK // GRP // 16):
                                  g * (CHUNK // GRP // 16) + ngi // 16],
                            num_idxs=ngi, num_idxs_reg=ngi, elem_size=H,
                            transpose=True, sbuf_tokens_per_rank=128,
                            sbuf_free_dim_per_rank=H * 2,
                            single_packet=False,
                            queue_num=0 if singleq else qctr[0] % 4,
                        )
                        qctr[0] += 1
                    for k in range(g, g + nb):
                        j0 = 0
                        i0 = (k - g) * (CHUNK // GRP)
                        xj = gpool.tile([128, CHUNK], dt.bfloat16, tag="xj")
                        if nogather:
                            nc.vector.memset(xj[:], 0.125)
                        else:
                            nc.gpsimd.dma_gather(
                                xj[:].rearrange("p (a b) -> p a b", a=1),
                                table[:],
                                srcidx[:, k * (CHUNK // 16):
                                       (k + 1) * (CHUNK // 16)],
                                num_idxs=CHUNK, num_idxs_reg=CHUNK,
                                elem_size=H,
                                transpose=True, sbuf_tokens_per_rank=128,
                                sbuf_free_dim_per_rank=H * 2,
                                single_packet=False,
                                queue_num=0 if singleq else qctr[0] % 4,
                            )
                            qctr[0] += 1
                        pre = psp.tile([128, CHUNK], dt.float32, tag="pre")
                        for b in range(CHUNK // 512):
                            nc.tensor.matmul(pre[:, ts(b, 512)], B,
                                             xj[:, j0 + b * 512:
                                                j0 + (b + 1) * 512],
                                             start=True, stop=False)
                        for b in range(CHUNK // 512):
                            xi_b = (xi[:, i0 + b * 64:i0 + (b + 1) * 64]
                                    .unsqueeze(2)
                                    .broadcast_to([128, 64, GRP]))
                            nc.tensor.matmul(pre[:, ts(b, 512)], A, xi_b,
                                             start=False, stop=True)
                        s = spool.tile([128, CHUNK], dt.bfloat16, tag="s")
                        nc.scalar.activation(s[:], pre[:], AF.Relu,
                                             bias=betas[:, l:l + 1], scale=1.0)
                        msg = psp.tile([128, CHUNK], dt.float32, tag="msg")
                        for b in range(CHUNK // 512):
                            nc.tensor.matmul(msg[:, ts(b, 512)], W2,
                                             s[:, ts(b, 512)],
                                             start=True, stop=True)
                        for (col0, cnt, d, off) in calls[k]:
                            nc.vector.reduce_max(
                                hfm[:, col0:col0 + cnt],
                                msg[:, off:off + cnt * d * GRP]
                                .rearrange("p (n e) -> p n e", e=d * GRP),
                                axis=mybir.AxisListType.X)

                if not last:
                    stage = stgp.tile([128, TOK_PER_CORE], dt.bfloat16,
                                      tag="stage")
                    for t in range(R_TILES):
                        tp = psp.tile([128, 128], dt.bfloat16, tag="msg")
                        nc.tensor.transpose(tp[:], hfm[:, ts(t, 128)], ident())
                        nc.scalar.copy(stage[:, ts(t, 128)], tp[:])
                    agin = dramp.tile([TOK_PER_CORE, 128], dt.bfloat16,
                                      tag="agin")
                    nc.sync.dma_start(
                        agin[:].rearrange("(q t) e -> q (t e)", t=R_TILES),
                        stage[:])
                    table = tabp.tile([128, NTOK], dt.bfloat16, tag="table")
                    if use_coll:
                        agout = agouts[li % 7]
                        nc.gpsimd.collective_compute(
                            "AllGather", mybir.AluOpType.bypass,
                            replica_groups=[list(range(ncores))],
                            ins=[agin.opt()], outs=[agout.ap()])
                        nc.sync.dma_start(
                            table[:].rearrange("q (c t e) -> q c t e",
                                               c=ncores, t=R_TILES),
                            agout.ap().rearrange("(c q t) e -> q c t e",
                                                 c=ncores, t=R_TILES))
                    else:
                        # debug: local-only table refresh (wrong values,
                        # exercise-only)
                        nc.sync.dma_start(
                            table[:, :TOK_PER_CORE]
                            .rearrange("q (t e) -> q t e", t=R_TILES),
                            agin[:].rearrange("(q t) e -> q t e", t=R_TILES))
                        nc.vector.memset(table[:, TOK_PER_CORE:], 0.0)
                else:
                    stage = stgp.tile([128, TOK_PER_CORE], dt.float32,
                                      tag="fstage")
                    Wf = wblk(24)
                    for t in range(R_TILES):
                        tp = psp.tile([128, 128], dt.float32, tag="msg")
                        nc.tensor.matmul(tp[:], hfm[:, ts(t, 128)], Wf,
                                         start=True, stop=True)
                        nc.scalar.copy(stage[:, ts(t, 128)], tp[:])
                    nc.sync.dma_start(
                        score_d.ap().rearrange("(q t) e -> q (t e)",
                                               t=R_TILES),
                        stage[:])
    nc.compile()
    return nc


# --------------------------------------------------------------------------
# entry point
# --------------------------------------------------------------------------

def _prep_inputs(inputs, ncores):
    W = fold_weights(inputs)
    plan = build_plan(np.asarray(inputs["edge_index"]), ncores)

    x = np.asarray(inputs["x"], np.float64)
    noise = np.asarray(inputs["noise"], np.float64)
    x_noisy = (W["a_coef"] * x + W["b_coef"] * noise).astype(np.float32)

    t0 = table_from_rows(x_noisy, plan["gidx_of_node"], ncores)

    wts = np.zeros((26, 128, 128), bf16)
    for l in range(8):
        wts[3 * l] = W["A"][l].astype(bf16)
        wts[3 * l + 1] = W["B"][l].astype(bf16)
        wts[3 * l + 2] = W["W2"][l].astype(bf16)
    wts[24] = W["Wf"].astype(bf16)
    wts[25] = np.eye(128).astype(bf16)
    betas = np.stack([b.astype(np.float32) for b in W["beta"]], axis=1)

    in_maps = []
    for c in range(ncores):
        in_maps.append({
            "t0": t0,
            "srcidx": idx_sbuf_layout(plan["src_streams"][c]),
            "xiidx": idx_sbuf_layout(plan["xi_streams"][c]),
            "wts": wts,
            "betas": np.ascontiguousarray(betas),
        })
    return W, plan, in_maps


def _finish(results, W, plan, noise, ncores):
    """Per-core score tiles -> MSE (float64 host reduction)."""
    score = np.zeros((N, H), np.float64)
    for c in range(ncores):
        sc = results[c]["score"]  # [1280,128] rows q*10+t for col=128t+q
        nodes = plan["core_nodes"][c]
        cols = plan["col_of_node"][nodes]
        rows = (cols % 128) * R_TILES + cols // 128
        score[nodes] = sc[rows].astype(np.float64)
    score += W["cf"][None, :]
    return np.float32(np.mean((score - np.asarray(noise, np.float64)) ** 2))


_CACHE = {}


def run(inputs, trace=False, trace_kwargs=None):
    from concourse.bass_utils import run_bass_kernel_spmd

    ncores = NCORES
    W, plan, in_maps = _prep_inputs(inputs, ncores)

    key = (plan["nchunk"], plan["ncols"])
    if key not in _CACHE:
        _CACHE[key] = build_nc(plan, ncores)
    nc = _CACHE[key]

    res = run_bass_kernel_spmd(nc, in_maps, core_ids=list(range(ncores)),
                               trace=trace, **(trace_kwargs or {}))
    return _finish(res.results, W, plan, inputs["noise"], ncores), res


def kernel(**inputs) -> np.ndarray:
    return run(inputs)[0]
